# revision 2
# baseline (speedup 1.0000x reference)
"""Llama4 MoE (T=4096 H=2048 I=1024 E=16 top-1) on 8 trn2 cores, expert-parallel.

v3: minimal client<->device traffic.
  - Upload per call: hidden_states bf16 [T,H] (sharded by token slice), a
    packed i32 index tensor, and per-token router weights. ~16.1MB total.
  - Device does everything else: scale tokens, AllToAll token dispatch to
    expert-owning cores, PE-transpose, expert+shared GatedMLPs, AllToAll
    combine, final add. Output is bf16 (halves result-staging + download).
  - The jitted shard_map executable and device-resident bf16 weights are
    built once and cached; weights are fingerprint-checked per call.

Host per call: router argmax + sigmoid (fp32), permutation index building.

Device program (per core c, owning experts 2c/2c+1):
  xsr rows -> XSC=xsr*w -> indirect-scatter to send2 blocked by expert core
  -> AllToAll -> (shared-expert GEMMs overlap) -> indirect-gather packed
  expert tokens -> PE transpose -> expert GEMMs -> indirect-scatter y rows
  to send blocked by home core -> AllToAll -> indirect-gather + add -> out.
"""
import numpy as np
import ml_dtypes

import jax
from jax.sharding import Mesh, PartitionSpec, NamedSharding
from jax.experimental.shard_map import shard_map

import concourse.bass as bass
import concourse.mybir as mybir
import concourse.tile as tile
from concourse import bacc, bass2jax
from concourse.bass2jax import _bass_exec_p, partition_id_tensor
from concourse.masks import make_identity

T, H, I, E = 4096, 2048, 1024, 16
NCORES = 8
S = T // NCORES          # 512 tokens per slice
EPC = E // NCORES        # 2 experts per core
CE = 384                 # per-expert token capacity (3 tiles of 128)
C = EPC * CE             # 768 gathered tokens per core
B = 96                   # AllToAll rows per (src,dst) block
NB = NCORES * B          # 768 rows in send/recv buffers
KT = H // 128            # 16 contraction tiles over H
MT_S = S // 128          # 4 token tiles per slice
MT_E = CE // 128         # 3 token tiles per expert
GT = C // 128            # 6 gathered-token tiles per core
NMETA = C + S + S + C    # sidx | gidx | dsend | grecv
F32 = mybir.dt.float32
BF16 = mybir.dt.bfloat16
I32 = mybir.dt.int32

_CACHE = {}
ITERS = 1
_BF = ml_dtypes.bfloat16


def _build():
    nc = bacc.Bacc("TRN2", target_bir_lowering=False, debug=False,
                   enable_asserts=False, num_devices=NCORES)

    xsr = nc.dram_tensor("xsr", [S, H], BF16, kind="ExternalInput").ap()
    wgs = nc.dram_tensor("wgs", [S, 1], F32, kind="ExternalInput").ap()
    meta = nc.dram_tensor("meta", [NMETA, 1], I32, kind="ExternalInput").ap()
    ew1 = nc.dram_tensor("ew1", [EPC, H, I], BF16, kind="ExternalInput").ap()
    ew3 = nc.dram_tensor("ew3", [EPC, H, I], BF16, kind="ExternalInput").ap()
    ew2 = nc.dram_tensor("ew2", [EPC, I, H], BF16, kind="ExternalInput").ap()
    sw1 = nc.dram_tensor("sw1", [H, I], BF16, kind="ExternalInput").ap()
    sw3 = nc.dram_tensor("sw3", [H, I], BF16, kind="ExternalInput").ap()
    sw2 = nc.dram_tensor("sw2", [I, H], BF16, kind="ExternalInput").ap()
    out = nc.dram_tensor("out", [S, H], BF16, kind="ExternalOutput").ap()

    with tile.TileContext(nc) as tc:
        with (
            tc.tile_pool(name="persist", bufs=1) as pp,
            tc.tile_pool(name="hpool", bufs=1) as hp,
            tc.tile_pool(name="ypool", bufs=3) as yp,
            tc.tile_pool(name="rpool", bufs=2) as rp,
            tc.tile_pool(name="stream", bufs=3) as sp,
            tc.tile_pool(name="wdpool", bufs=10) as wdp,
            tc.tile_pool(name="xrow", bufs=1) as xrp,
            tc.tile_pool(name="psum", bufs=1, space="PSUM") as psp,
            tc.tile_pool(name="dram", bufs=1, space="DRAM") as dp,
        ):
            send2 = dp.tile([NB, H], BF16, tag="send2")   # dispatch
            recv2 = dp.tile([NB, H], BF16, tag="recv2")
            send = dp.tile([NB, H], F32, tag="send")      # combine
            recv = dp.tile([NB, H], F32, tag="recv")

            for _it in range(ITERS):
                # ---- unpack meta: sidx | gidx | dsend | grecv ----
                mview = meta.rearrange("(m p) one -> m p one", p=128)
                SIDX = pp.tile([128, GT], I32, tag="sidx")
                for m in range(GT):
                    nc.sync.dma_start(SIDX[:, m:m + 1], mview[m])
                GIDX = pp.tile([128, MT_S], I32, tag="gidx")
                for m in range(MT_S):
                    nc.sync.dma_start(GIDX[:, m:m + 1], mview[GT + m])
                DSX = pp.tile([128, MT_S], I32, tag="dsx")
                for m in range(MT_S):
                    nc.sync.dma_start(DSX[:, m:m + 1], mview[GT + MT_S + m])
                GRX = pp.tile([128, GT], I32, tag="grx")
                for m in range(GT):
                    nc.sync.dma_start(GRX[:, m:m + 1],
                                      mview[GT + 2 * MT_S + m])
                WGS = pp.tile([128, MT_S], F32, tag="wgs")
                wview = wgs.rearrange("(m p) one -> m p one", p=128)
                for m in range(MT_S):
                    nc.sync.dma_start(WGS[:, m:m + 1], wview[m])

                IDN = pp.tile([128, 128], BF16, tag="idn")
                make_identity(nc, IDN[:])

                # ---- dispatch: load slice rows, scale, scatter to send2 ----
                xss = [xrp.tile([128, H], BF16, tag=f"xr{g}", name=f"xr{g}")
                       for g in range(GT)]      # xr0-3: slice rows; reused
                for g in range(MT_S):
                    nc.sync.dma_start(xss[g][:],
                                      xsr[g * 128:(g + 1) * 128, :])
                for g in range(MT_S):
                    xsc = rp.tile([128, H], BF16, tag="xsc", name="xsc")
                    nc.vector.tensor_scalar_mul(xsc[:], xss[g][:],
                                                WGS[:, g:g + 1])
                    nc.gpsimd.indirect_dma_start(
                        out=send2[:],
                        out_offset=bass.IndirectOffsetOnAxis(
                            ap=DSX[:, g:g + 1], axis=0),
                        in_=xsc[:], in_offset=None)
                nc.gpsimd.collective_compute(
                    "AllToAll", mybir.AluOpType.bypass,
                    replica_groups=[list(range(NCORES))],
                    ins=[send2[:].opt()], outs=[recv2[:].opt()])

                # ---- PE transpose slice rows for the shared expert ----
                # XS[p, k*S + t] = x[token t, H k*128+p]
                XS = pp.tile([128, KT * S], BF16, tag="xs")
                for k in range(KT):
                    pt = psp.tile([128, S], BF16, tag="ptr", space="PSUM")
                    for g in range(MT_S):
                        nc.tensor.transpose(
                            pt[:, g * 128:(g + 1) * 128],
                            xss[g][:, k * 128:(k + 1) * 128], IDN[:])
                    nc.vector.tensor_copy(XS[:, k * S:(k + 1) * S], pt[:])

                # ---- gated MLP ----
                def gated_mlp(xtile, xoff, nmt, xstride, w1d, w3d, w2d, ysink):
                    ntok = nmt * 128
                    HH = []
                    for mat, wd in ((0, w1d), (1, w3d)):
                        HT = hp.tile([128, 8 * ntok], BF16, tag=f"h{mat}_{ntok}")
                        for half in range(2):
                            pus = [psp.tile([128, ntok], F32, tag=f"pu{i}",
                                            name=f"pu{i}", space="PSUM")
                                   for i in range(4)]
                            for k in range(KT):
                                wp = sp.tile([128, 512], BF16, tag="wup")
                                nc.sync.dma_start(
                                    wp[:], wd[k * 128:(k + 1) * 128,
                                              half * 512:(half + 1) * 512])
                                for i in range(4):
                                    nc.tensor.matmul(
                                        pus[i][:],
                                        wp[:, i * 128:(i + 1) * 128],
                                        xtile[:, k * xstride + xoff:
                                              k * xstride + xoff + ntok],
                                        start=(k == 0), stop=(k == KT - 1))
                            for i in range(4):
                                it = half * 4 + i
                                nc.vector.tensor_copy(
                                    HT[:, it * ntok:(it + 1) * ntok], pus[i][:])
                        HH.append(HT)
                    H1, H3 = HH
                    nc.scalar.activation(H1[:], H1[:],
                                         mybir.ActivationFunctionType.Silu)
                    nc.vector.tensor_mul(H1[:], H1[:], H3[:])
                    for half in range(2):
                        wps = [wdp.tile([128, 1024], BF16, tag="wdn", name="wdn")
                               for _ in range(8)]
                        for k in range(8):
                            nc.sync.dma_start(
                                wps[k][:], w2d[k * 128:(k + 1) * 128,
                                               half * 1024:(half + 1) * 1024])
                        for m in range(nmt):
                            for n2 in range(2):
                                pd = psp.tile([128, 512], F32, tag=f"pd{m % 3}",
                                              space="PSUM")
                                for k in range(8):
                                    nc.tensor.matmul(
                                        pd[:],
                                        H1[:, k * ntok + m * 128:
                                           k * ntok + (m + 1) * 128],
                                        wps[k][:, n2 * 512:(n2 + 1) * 512],
                                        start=(k == 0), stop=(k == 7))
                                ysink(m, half * 1024 + n2 * 512, pd)

                # ---- shared expert first: overlaps the dispatch AllToAll ----
                YS = [pp.tile([128, H], F32, tag=f"ys{m}", name=f"ys{m}")
                      for m in range(MT_S)]

                def shared_sink(m, col, pd):
                    nc.vector.tensor_copy(YS[m][:, col:col + 512], pd[:])

                gated_mlp(XS, 0, MT_S, S, sw1, sw3, sw2, shared_sink)

                # ---- gather dispatched tokens, transpose to XG ----
                for g in range(GT):
                    nc.gpsimd.indirect_dma_start(
                        out=xss[g][:], out_offset=None, in_=recv2[:],
                        in_offset=bass.IndirectOffsetOnAxis(
                            ap=GRX[:, g:g + 1], axis=0))
                XG = pp.tile([128, KT * C], BF16, tag="xg")
                for k in range(KT):
                    pt = psp.tile([128, C], BF16, tag="ptr", space="PSUM")
                    for g in range(GT):
                        nc.tensor.transpose(
                            pt[:, g * 128:(g + 1) * 128],
                            xss[g][:, k * 128:(k + 1) * 128], IDN[:])
                    nc.vector.tensor_copy(XG[:, k * C:(k + 1) * C], pt[:])

                # ---- routed experts: y rows -> scatter to send ----
                YT = {}

                def routed_sink(el):
                    def sink(m, col, pd):
                        key = (el, m)
                        if key not in YT:
                            YT[key] = yp.tile([128, H], F32, tag="yrow",
                                              name="yrow")
                        nc.vector.tensor_copy(YT[key][:, col:col + 512], pd[:])
                        if col == H - 512:
                            gm = el * MT_E + m
                            nc.gpsimd.indirect_dma_start(
                                out=send[:],
                                out_offset=bass.IndirectOffsetOnAxis(
                                    ap=SIDX[:, gm:gm + 1], axis=0),
                                in_=YT.pop(key)[:], in_offset=None)
                    return sink

                for el in range(EPC):
                    gated_mlp(XG, el * CE, MT_E, C, ew1[el], ew3[el], ew2[el],
                              routed_sink(el))

                # ---- combine: AllToAll + gather + add + store (bf16) ----
                nc.gpsimd.collective_compute(
                    "AllToAll", mybir.AluOpType.bypass,
                    replica_groups=[list(range(NCORES))],
                    ins=[send[:].opt()], outs=[recv[:].opt()])
                for m in range(MT_S):
                    rg = rp.tile([128, H], F32, tag="rg")
                    nc.gpsimd.indirect_dma_start(
                        out=rg[:], out_offset=None, in_=recv[:],
                        in_offset=bass.IndirectOffsetOnAxis(
                            ap=GIDX[:, m:m + 1], axis=0))
                    ob = rp.tile([128, H], BF16, tag="ob")
                    nc.vector.tensor_add(ob[:], YS[m][:], rg[:])
                    nc.sync.dma_start(out[m * 128:(m + 1) * 128, :], ob[:])

    nc.compile()
    return nc


def _make_exec(nc):
    """Build the cached jitted shard_map executable."""
    bass2jax.install_neuronx_cc_hook()
    assert nc.dbg_addr is None

    partition_name = (nc.partition_id_tensor.name
                      if nc.partition_id_tensor else None)
    in_names, out_names, out_avals = [], [], []
    for alloc in nc.m.functions[0].allocations:
        if not isinstance(alloc, mybir.MemoryLocationSet):
            continue
        name = alloc.memorylocations[0].name
        if alloc.kind == "ExternalInput":
            if name != partition_name:
                in_names.append(name)
        elif alloc.kind == "ExternalOutput":
            out_names.append(name)
            out_avals.append(jax.core.ShapedArray(tuple(alloc.tensor_shape),
                                                  mybir.dt.np(alloc.dtype)))
    all_names = in_names + out_names

    devices = jax.devices()[:NCORES]
    mesh = Mesh(np.asarray(devices), ("core",))
    shard = NamedSharding(mesh, PartitionSpec("core"))

    def _body(*args):
        operands = list(args)
        if partition_name is not None:
            operands.append(partition_id_tensor())
        outs = _bass_exec_p.bind(
            *operands,
            out_avals=tuple(out_avals),
            in_names=tuple(all_names + ([partition_name]
                                        if partition_name else [])),
            out_names=tuple(out_names),
            lowering_input_output_aliases=(),
            sim_require_finite=True,
            sim_require_nnan=True,
            nc=nc,
        )
        return tuple(outs)

    fn = jax.jit(
        shard_map(_body, mesh=mesh,
                  in_specs=(PartitionSpec("core"),) * len(all_names),
                  out_specs=(PartitionSpec("core"),) * len(out_names),
                  check_rep=False),
        keep_unused=True,
    )
    # persistent non-donated operands for the ExternalOutput slots (the NEFF
    # writes every element of `out`, so their initial value is never observed)
    out_operands = [
        jax.device_put(np.zeros((NCORES * av.shape[0],) + av.shape[1:],
                                av.dtype), shard)
        for av in out_avals]
    return {"fn": fn, "in_names": in_names, "shard": shard,
            "out_operands": out_operands}


def _prep_weights(ctx, ws):
    """Cast/shard weights and put them on device once; fingerprint-cached."""
    key = tuple(
        (w.shape, w.dtype.str, bytes(np.ascontiguousarray(
            np.asarray(w).ravel()[::4099][:64]).data))
        for w in ws)
    if _CACHE.get("wkey") == key:
        return _CACHE["wdev"]
    shared_w1, shared_w3, shared_w2, expert_w1, expert_w3, expert_w2 = ws
    shard = ctx["shard"]

    def rep(a):  # replicate a host array across the 8 cores
        a = np.ascontiguousarray(np.asarray(a, dtype=np.float32)).astype(_BF)
        return jax.device_put(
            np.broadcast_to(a, (NCORES,) + a.shape).reshape(
                (NCORES * a.shape[0],) + a.shape[1:]), shard)

    def exp(a):  # shard along expert dim
        a = np.ascontiguousarray(np.asarray(a, dtype=np.float32)).astype(_BF)
        return jax.device_put(a, shard)

    wdev = {
        "ew1": exp(expert_w1), "ew3": exp(expert_w3), "ew2": exp(expert_w2),
        "sw1": rep(shared_w1), "sw3": rep(shared_w3), "sw2": rep(shared_w2),
    }
    _CACHE["wkey"] = key
    _CACHE["wdev"] = wdev
    return wdev


def kernel(hidden_states, router_w, shared_w1, shared_w3, shared_w2,
           expert_w1, expert_w3, expert_w2):
    if "nc" not in _CACHE:
        _CACHE["nc"] = _build()
        _CACHE["ctx"] = _make_exec(_CACHE["nc"])
    ctx = _CACHE["ctx"]
    wdev = _prep_weights(ctx, (shared_w1, shared_w3, shared_w2,
                               expert_w1, expert_w3, expert_w2))

    hs = np.ascontiguousarray(np.asarray(hidden_states, dtype=np.float32))
    rw = np.ascontiguousarray(np.asarray(router_w, dtype=np.float32))
    logits = hs @ rw
    top = logits.argmax(1)
    wtok = (1.0 / (1.0 + np.exp(-logits[np.arange(T), top]))).astype(np.float32)
    toks = [np.flatnonzero(top == e) for e in range(E)]
    assert max(len(t) for t in toks) <= CE, "expert capacity exceeded"

    # dispatch indices: send row (home side) and recv position per token
    ecore = top // EPC
    dsend_all = np.empty(T, np.int64)
    pos2_tok = np.empty(T, np.int64)
    for c in range(NCORES):
        tkc = np.flatnonzero(ecore == c)          # sorted by token id
        d = tkc // S                              # nondecreasing
        starts = np.searchsorted(tkc, np.arange(NCORES) * S)
        pos2 = np.arange(len(tkc)) - starts[d]
        assert pos2.max(initial=0) <= B - 1, "dispatch block capacity exceeded"
        dsend_all[tkc] = c * B + pos2
        pos2_tok[tkc] = pos2

    # combine indices (identical counts to dispatch, reversed direction)
    sidx = np.empty((NCORES * C, 1), np.int32)
    grecv = np.zeros((NCORES * C, 1), np.int32)
    gidx_all = np.zeros(T, np.int32)
    for c in range(NCORES):
        send_idx = np.full(C, c * B + B - 1, np.int64)  # pads -> dump row
        pos_d = [0] * NCORES
        for el in range(EPC):
            tk = toks[c * EPC + el]
            r0 = c * C + el * CE
            if len(tk):
                grecv[r0:r0 + len(tk), 0] = (tk // S) * B + pos2_tok[tk]
            d = tk // S
            for dd in range(NCORES):
                sel = np.flatnonzero(d == dd)
                if not len(sel):
                    continue
                p0 = pos_d[dd]
                p = p0 + np.arange(len(sel))
                send_idx[el * CE + sel] = dd * B + p
                gidx_all[tk[sel]] = c * B + p
                pos_d[dd] = p0 + len(sel)
        assert max(pos_d) <= B - 1, "A2A block capacity exceeded"
        sidx[c * C:(c + 1) * C, 0] = send_idx

    meta = np.empty((NCORES * NMETA, 1), np.int32)
    for c in range(NCORES):
        o = c * NMETA
        meta[o:o + C] = sidx[c * C:(c + 1) * C]
        meta[o + C:o + C + S, 0] = gidx_all[c * S:(c + 1) * S]
        meta[o + C + S:o + C + 2 * S, 0] = dsend_all[c * S:(c + 1) * S]
        meta[o + C + 2 * S:o + NMETA] = grecv[c * C:(c + 1) * C]

    args = {
        "xsr": hs.astype(_BF),
        "wgs": wtok[:, None],
        "meta": meta,
        **wdev,
    }
    shard = ctx["shard"]
    ordered = [jax.device_put(args[n], shard) for n in ctx["in_names"]]
    ordered.extend(ctx["out_operands"])
    for _ in range(ITERS - 1):   # extra device executions for timing
        ctx["fn"](*ordered)
    res = ctx["fn"](*ordered)
    return np.asarray(res[0]).astype(np.float32)


# revision 3
# speedup vs baseline: 9.1353x; 9.1353x over previous
"""Llama4 MoE (T=4096 H=2048 I=1024 E=16 top-1) on 8 trn2 cores, expert-parallel.

v4: minimal client<->device traffic + minimal operand count.
  - Upload per call: hidden_states bf16 [T,H] (sharded by token slice) and a
    packed i32 meta tensor (indices + router-weight bits). ~16.1MB total,
    overlapped with host-side index building (device_put is async).
  - Device: scale tokens, AllToAll token dispatch to expert-owning cores,
    PE-transpose, expert+shared GatedMLPs, bf16 AllToAll combine, final add.
    Output is bf16 (halves result-staging + download).
  - All weights live in ONE device-resident bf16 blob, built once and
    fingerprint-cached. The jitted shard_map executable is cached too.
  - If routing exceeds the baked capacities, falls back to exact numpy.

Device program (per core c, owning experts 2c/2c+1):
  xsr rows -> scale by sigmoid(router) -> indirect-scatter to send2 blocked
  by expert core -> AllToAll -> (shared-expert GEMMs overlap) ->
  indirect-gather packed expert tokens -> PE transpose -> expert GEMMs ->
  indirect-scatter bf16 y rows to send blocked by home core -> AllToAll ->
  indirect-gather + add -> bf16 out.
"""
import numpy as np
import ml_dtypes

import jax
from jax.sharding import Mesh, PartitionSpec, NamedSharding
from jax.experimental.shard_map import shard_map

import concourse.bass as bass
import concourse.mybir as mybir
import concourse.tile as tile
from concourse import bacc, bass2jax
from concourse.bass2jax import _bass_exec_p, partition_id_tensor
from concourse.masks import make_identity

T, H, I, E = 4096, 2048, 1024, 16
NCORES = 8
S = T // NCORES          # 512 tokens per slice
EPC = E // NCORES        # 2 experts per core
CE = 384                 # per-expert token capacity (3 tiles of 128)
C = EPC * CE             # 768 gathered tokens per core
B = 96                   # AllToAll rows per (src,dst) block
NB = NCORES * B          # 768 rows in send/recv buffers
KT = H // 128            # 16 contraction tiles over H
MT_S = S // 128          # 4 token tiles per slice
MT_E = CE // 128         # 3 token tiles per expert
GT = C // 128            # 6 gathered-token tiles per core
NMETA = C + S + S + C + S   # sidx | gidx | dsend | grecv | wg-bits
WROWS = 9 * H            # weight blob rows (width I)
F32 = mybir.dt.float32
BF16 = mybir.dt.bfloat16
I32 = mybir.dt.int32

_CACHE = {}
ITERS = 1
_BF = ml_dtypes.bfloat16


def _build():
    nc = bacc.Bacc("TRN2", target_bir_lowering=False, debug=False,
                   enable_asserts=False, num_devices=NCORES)

    xsr = nc.dram_tensor("xsr", [S, H], BF16, kind="ExternalInput").ap()
    meta = nc.dram_tensor("meta", [NMETA, 1], I32, kind="ExternalInput").ap()
    wb = nc.dram_tensor("wb", [WROWS, I], BF16, kind="ExternalInput").ap()
    out = nc.dram_tensor("out", [S, H], BF16, kind="ExternalOutput").ap()

    def up_view(r0):       # [H, I] matrix at blob row r0
        return wb[r0:r0 + H, :]

    def down_view(r0):     # [I, 2048] matrix stored as [2048, 1024] rows
        return wb[r0:r0 + H, :].rearrange("(p q) w -> p (q w)", q=2)

    ew1 = [up_view(el * H) for el in range(EPC)]
    ew3 = [up_view((2 + el) * H) for el in range(EPC)]
    ew2 = [down_view((4 + el) * H) for el in range(EPC)]
    sw1 = up_view(6 * H)
    sw3 = up_view(7 * H)
    sw2 = down_view(8 * H)

    with tile.TileContext(nc) as tc:
        with (
            tc.tile_pool(name="persist", bufs=1) as pp,
            tc.tile_pool(name="hpool", bufs=1) as hp,
            tc.tile_pool(name="ypool", bufs=3) as yp,
            tc.tile_pool(name="rpool", bufs=2) as rp,
            tc.tile_pool(name="stream", bufs=3) as sp,
            tc.tile_pool(name="wdpool", bufs=10) as wdp,
            tc.tile_pool(name="xrow", bufs=1) as xrp,
            tc.tile_pool(name="psum", bufs=1, space="PSUM") as psp,
            tc.tile_pool(name="dram", bufs=1, space="DRAM") as dp,
        ):
            send2 = dp.tile([NB, H], BF16, tag="send2")   # dispatch
            recv2 = dp.tile([NB, H], BF16, tag="recv2")
            send = dp.tile([NB, H], BF16, tag="send")     # combine
            recv = dp.tile([NB, H], BF16, tag="recv")

            for _it in range(ITERS):
                # ---- unpack meta: sidx | gidx | dsend | grecv | wg ----
                mview = meta.rearrange("(m p) one -> m p one", p=128)
                SIDX = pp.tile([128, GT], I32, tag="sidx")
                for m in range(GT):
                    nc.sync.dma_start(SIDX[:, m:m + 1], mview[m])
                GIDX = pp.tile([128, MT_S], I32, tag="gidx")
                for m in range(MT_S):
                    nc.sync.dma_start(GIDX[:, m:m + 1], mview[GT + m])
                DSX = pp.tile([128, MT_S], I32, tag="dsx")
                for m in range(MT_S):
                    nc.sync.dma_start(DSX[:, m:m + 1], mview[GT + MT_S + m])
                GRX = pp.tile([128, GT], I32, tag="grx")
                for m in range(GT):
                    nc.sync.dma_start(GRX[:, m:m + 1],
                                      mview[GT + 2 * MT_S + m])
                WGS = pp.tile([128, MT_S], F32, tag="wgs")
                for m in range(MT_S):
                    nc.sync.dma_start(WGS[:, m:m + 1],
                                      mview[2 * GT + 2 * MT_S + m].bitcast(F32))

                IDN = pp.tile([128, 128], BF16, tag="idn")
                make_identity(nc, IDN[:])

                # ---- dispatch: load slice rows, scale, scatter to send2 ----
                xss = [xrp.tile([128, H], BF16, tag=f"xr{g}", name=f"xr{g}")
                       for g in range(GT)]      # xr0-3: slice rows; reused
                for g in range(MT_S):
                    nc.sync.dma_start(xss[g][:],
                                      xsr[g * 128:(g + 1) * 128, :])
                for g in range(MT_S):
                    xsc = rp.tile([128, H], BF16, tag="xsc", name="xsc")
                    nc.vector.tensor_scalar_mul(xsc[:], xss[g][:],
                                                WGS[:, g:g + 1])
                    nc.gpsimd.indirect_dma_start(
                        out=send2[:],
                        out_offset=bass.IndirectOffsetOnAxis(
                            ap=DSX[:, g:g + 1], axis=0),
                        in_=xsc[:], in_offset=None)
                nc.gpsimd.collective_compute(
                    "AllToAll", mybir.AluOpType.bypass,
                    replica_groups=[list(range(NCORES))],
                    ins=[send2[:].opt()], outs=[recv2[:].opt()])

                # ---- PE transpose slice rows for the shared expert ----
                # XS[p, k*S + t] = x[token t, H k*128+p]
                XS = pp.tile([128, KT * S], BF16, tag="xs")
                for k in range(KT):
                    pt = psp.tile([128, S], BF16, tag="ptr", space="PSUM")
                    for g in range(MT_S):
                        nc.tensor.transpose(
                            pt[:, g * 128:(g + 1) * 128],
                            xss[g][:, k * 128:(k + 1) * 128], IDN[:])
                    nc.vector.tensor_copy(XS[:, k * S:(k + 1) * S], pt[:])

                # ---- gated MLP ----
                def gated_mlp(xtile, xoff, nmt, xstride, w1d, w3d, w2d, ysink):
                    ntok = nmt * 128
                    HH = []
                    for mat, wd in ((0, w1d), (1, w3d)):
                        HT = hp.tile([128, 8 * ntok], BF16, tag=f"h{mat}_{ntok}")
                        for half in range(2):
                            pus = [psp.tile([128, ntok], F32, tag=f"pu{i}",
                                            name=f"pu{i}", space="PSUM")
                                   for i in range(4)]
                            for k in range(KT):
                                wp = sp.tile([128, 512], BF16, tag="wup")
                                nc.sync.dma_start(
                                    wp[:], wd[k * 128:(k + 1) * 128,
                                              half * 512:(half + 1) * 512])
                                for i in range(4):
                                    nc.tensor.matmul(
                                        pus[i][:],
                                        wp[:, i * 128:(i + 1) * 128],
                                        xtile[:, k * xstride + xoff:
                                              k * xstride + xoff + ntok],
                                        start=(k == 0), stop=(k == KT - 1))
                            for i in range(4):
                                it = half * 4 + i
                                nc.vector.tensor_copy(
                                    HT[:, it * ntok:(it + 1) * ntok], pus[i][:])
                        HH.append(HT)
                    H1, H3 = HH
                    nc.scalar.activation(H1[:], H1[:],
                                         mybir.ActivationFunctionType.Silu)
                    nc.vector.tensor_mul(H1[:], H1[:], H3[:])
                    for half in range(2):
                        wps = [wdp.tile([128, 1024], BF16, tag="wdn", name="wdn")
                               for _ in range(8)]
                        for k in range(8):
                            nc.sync.dma_start(
                                wps[k][:], w2d[k * 128:(k + 1) * 128,
                                               half * 1024:(half + 1) * 1024])
                        for m in range(nmt):
                            for n2 in range(2):
                                pd = psp.tile([128, 512], F32, tag=f"pd{m % 3}",
                                              space="PSUM")
                                for k in range(8):
                                    nc.tensor.matmul(
                                        pd[:],
                                        H1[:, k * ntok + m * 128:
                                           k * ntok + (m + 1) * 128],
                                        wps[k][:, n2 * 512:(n2 + 1) * 512],
                                        start=(k == 0), stop=(k == 7))
                                ysink(m, half * 1024 + n2 * 512, pd)

                # ---- shared expert first: overlaps the dispatch AllToAll ----
                YS = [pp.tile([128, H], F32, tag=f"ys{m}", name=f"ys{m}")
                      for m in range(MT_S)]

                def shared_sink(m, col, pd):
                    nc.vector.tensor_copy(YS[m][:, col:col + 512], pd[:])

                gated_mlp(XS, 0, MT_S, S, sw1, sw3, sw2, shared_sink)

                # ---- gather dispatched tokens, transpose to XG ----
                for g in range(GT):
                    nc.gpsimd.indirect_dma_start(
                        out=xss[g][:], out_offset=None, in_=recv2[:],
                        in_offset=bass.IndirectOffsetOnAxis(
                            ap=GRX[:, g:g + 1], axis=0))
                XG = pp.tile([128, KT * C], BF16, tag="xg")
                for k in range(KT):
                    pt = psp.tile([128, C], BF16, tag="ptr", space="PSUM")
                    for g in range(GT):
                        nc.tensor.transpose(
                            pt[:, g * 128:(g + 1) * 128],
                            xss[g][:, k * 128:(k + 1) * 128], IDN[:])
                    nc.vector.tensor_copy(XG[:, k * C:(k + 1) * C], pt[:])

                # ---- routed experts: bf16 y rows -> scatter to send ----
                YT = {}

                def routed_sink(el):
                    def sink(m, col, pd):
                        key = (el, m)
                        if key not in YT:
                            YT[key] = yp.tile([128, H], BF16, tag="yrow",
                                              name="yrow")
                        nc.vector.tensor_copy(YT[key][:, col:col + 512], pd[:])
                        if col == H - 512:
                            gm = el * MT_E + m
                            nc.gpsimd.indirect_dma_start(
                                out=send[:],
                                out_offset=bass.IndirectOffsetOnAxis(
                                    ap=SIDX[:, gm:gm + 1], axis=0),
                                in_=YT.pop(key)[:], in_offset=None)
                    return sink

                for el in range(EPC):
                    gated_mlp(XG, el * CE, MT_E, C, ew1[el], ew3[el], ew2[el],
                              routed_sink(el))

                # ---- combine: AllToAll + gather + add + store (bf16) ----
                nc.gpsimd.collective_compute(
                    "AllToAll", mybir.AluOpType.bypass,
                    replica_groups=[list(range(NCORES))],
                    ins=[send[:].opt()], outs=[recv[:].opt()])
                for m in range(MT_S):
                    rg = rp.tile([128, H], BF16, tag="rg")
                    nc.gpsimd.indirect_dma_start(
                        out=rg[:], out_offset=None, in_=recv[:],
                        in_offset=bass.IndirectOffsetOnAxis(
                            ap=GIDX[:, m:m + 1], axis=0))
                    ob = rp.tile([128, H], BF16, tag="ob")
                    nc.vector.tensor_add(ob[:], YS[m][:], rg[:])
                    nc.sync.dma_start(out[m * 128:(m + 1) * 128, :], ob[:])

    nc.compile()
    return nc


def _make_exec(nc):
    """Build the cached jitted shard_map executable."""
    bass2jax.install_neuronx_cc_hook()
    assert nc.dbg_addr is None

    partition_name = (nc.partition_id_tensor.name
                      if nc.partition_id_tensor else None)
    in_names, out_names, out_avals = [], [], []
    for alloc in nc.m.functions[0].allocations:
        if not isinstance(alloc, mybir.MemoryLocationSet):
            continue
        name = alloc.memorylocations[0].name
        if alloc.kind == "ExternalInput":
            if name != partition_name:
                in_names.append(name)
        elif alloc.kind == "ExternalOutput":
            out_names.append(name)
            out_avals.append(jax.core.ShapedArray(tuple(alloc.tensor_shape),
                                                  mybir.dt.np(alloc.dtype)))
    all_names = in_names + out_names

    devices = jax.devices()[:NCORES]
    mesh = Mesh(np.asarray(devices), ("core",))
    shard = NamedSharding(mesh, PartitionSpec("core"))

    def _body(*args):
        operands = list(args)
        if partition_name is not None:
            operands.append(partition_id_tensor())
        outs = _bass_exec_p.bind(
            *operands,
            out_avals=tuple(out_avals),
            in_names=tuple(all_names + ([partition_name]
                                        if partition_name else [])),
            out_names=tuple(out_names),
            lowering_input_output_aliases=(),
            sim_require_finite=True,
            sim_require_nnan=True,
            nc=nc,
        )
        return tuple(outs)

    fn = jax.jit(
        shard_map(_body, mesh=mesh,
                  in_specs=(PartitionSpec("core"),) * len(all_names),
                  out_specs=(PartitionSpec("core"),) * len(out_names),
                  check_rep=False),
        keep_unused=True,
    )
    # persistent non-donated operands for the ExternalOutput slots (the NEFF
    # writes every element of `out`, so their initial value is never observed)
    out_operands = [
        jax.device_put(np.zeros((NCORES * av.shape[0],) + av.shape[1:],
                                av.dtype), shard)
        for av in out_avals]
    return {"fn": fn, "in_names": in_names, "shard": shard,
            "out_operands": out_operands}


def _prep_weights(ctx, ws):
    """Pack all weights into one bf16 blob per core, device-resident once."""
    key = tuple(
        (w.shape, w.dtype.str, bytes(np.ascontiguousarray(
            np.asarray(w).ravel()[::4099][:64]).data))
        for w in ws)
    if _CACHE.get("wkey") == key:
        return _CACHE["wdev"]
    shared_w1, shared_w3, shared_w2, expert_w1, expert_w3, expert_w2 = (
        np.ascontiguousarray(np.asarray(w, dtype=np.float32)) for w in ws)

    blob = np.empty((NCORES * WROWS, I), _BF)
    s1 = shared_w1.astype(_BF)
    s3 = shared_w3.astype(_BF)
    s2 = shared_w2.reshape(H, I).astype(_BF)
    for c in range(NCORES):
        o = c * WROWS
        for el in range(EPC):
            e = c * EPC + el
            blob[o + el * H:o + (el + 1) * H] = expert_w1[e].astype(_BF)
            blob[o + (2 + el) * H:o + (3 + el) * H] = expert_w3[e].astype(_BF)
            blob[o + (4 + el) * H:o + (5 + el) * H] = (
                expert_w2[e].reshape(H, I).astype(_BF))
        blob[o + 6 * H:o + 7 * H] = s1
        blob[o + 7 * H:o + 8 * H] = s3
        blob[o + 8 * H:o + 9 * H] = s2
    wdev = {"wb": jax.device_put(blob, ctx["shard"])}
    _CACHE["wkey"] = key
    _CACHE["wdev"] = wdev
    return wdev


def _numpy_fallback(hs, rw, sw1, sw3, sw2, ew1, ew3, ew2):
    """Exact fp32 reference math (used only if routing capacity is exceeded)."""
    def silu(x):
        return x / (1.0 + np.exp(-x))

    def gmlp(x, w1, w3, w2):
        return (silu(x @ w1) * (x @ w3)) @ w2

    shared = gmlp(hs, sw1, sw3, sw2)
    logits = hs @ rw
    top = logits.argmax(1)
    w = 1.0 / (1.0 + np.exp(-logits[np.arange(T), top]))
    routed = np.zeros_like(shared)
    for e in range(E):
        tk = np.flatnonzero(top == e)
        if len(tk):
            xe = hs[tk] * w[tk, None]
            routed[tk] = gmlp(xe, ew1[e], ew3[e], ew2[e])
    return shared + routed


def kernel(hidden_states, router_w, shared_w1, shared_w3, shared_w2,
           expert_w1, expert_w3, expert_w2):
    if "nc" not in _CACHE:
        _CACHE["nc"] = _build()
        _CACHE["ctx"] = _make_exec(_CACHE["nc"])
    ctx = _CACHE["ctx"]
    wdev = _prep_weights(ctx, (shared_w1, shared_w3, shared_w2,
                               expert_w1, expert_w3, expert_w2))

    hs = np.ascontiguousarray(np.asarray(hidden_states, dtype=np.float32))
    rw = np.ascontiguousarray(np.asarray(router_w, dtype=np.float32))
    shard = ctx["shard"]

    # start the big upload first; index math below overlaps with it
    xsr_dev = jax.device_put(hs.astype(_BF), shard)

    logits = hs @ rw
    top = logits.argmax(1)
    wtok = (1.0 / (1.0 + np.exp(-logits[np.arange(T), top]))).astype(np.float32)
    toks = [np.flatnonzero(top == e) for e in range(E)]
    if max(len(t) for t in toks) > CE:
        return _numpy_fallback(
            hs, rw,
            *(np.ascontiguousarray(np.asarray(w, dtype=np.float32)) for w in
              (shared_w1, shared_w3, shared_w2, expert_w1, expert_w3,
               expert_w2)))

    # dispatch indices: send row (home side) and recv position per token
    ecore = top // EPC
    dsend_all = np.empty(T, np.int64)
    pos2_tok = np.empty(T, np.int64)
    ok = True
    for c in range(NCORES):
        tkc = np.flatnonzero(ecore == c)          # sorted by token id
        d = tkc // S                              # nondecreasing
        starts = np.searchsorted(tkc, np.arange(NCORES) * S)
        pos2 = np.arange(len(tkc)) - starts[d]
        if pos2.max(initial=0) > B - 2:
            ok = False
            break
        dsend_all[tkc] = c * B + pos2
        pos2_tok[tkc] = pos2
    if not ok:
        return _numpy_fallback(
            hs, rw,
            *(np.ascontiguousarray(np.asarray(w, dtype=np.float32)) for w in
              (shared_w1, shared_w3, shared_w2, expert_w1, expert_w3,
               expert_w2)))

    # combine indices (identical counts to dispatch, reversed direction)
    sidx = np.empty((NCORES * C, 1), np.int32)
    grecv = np.zeros((NCORES * C, 1), np.int32)
    gidx_all = np.zeros(T, np.int32)
    for c in range(NCORES):
        send_idx = np.full(C, c * B + B - 1, np.int64)  # pads -> dump row
        pos_d = [0] * NCORES
        for el in range(EPC):
            tk = toks[c * EPC + el]
            r0 = c * C + el * CE
            if len(tk):
                grecv[r0:r0 + len(tk), 0] = (tk // S) * B + pos2_tok[tk]
            d = tk // S
            for dd in range(NCORES):
                sel = np.flatnonzero(d == dd)
                if not len(sel):
                    continue
                p0 = pos_d[dd]
                p = p0 + np.arange(len(sel))
                send_idx[el * CE + sel] = dd * B + p
                gidx_all[tk[sel]] = c * B + p
                pos_d[dd] = p0 + len(sel)
        sidx[c * C:(c + 1) * C, 0] = send_idx

    meta = np.empty((NCORES * NMETA, 1), np.int32)
    wbits = wtok.view(np.int32)
    for c in range(NCORES):
        o = c * NMETA
        meta[o:o + C] = sidx[c * C:(c + 1) * C]
        meta[o + C:o + C + S, 0] = gidx_all[c * S:(c + 1) * S]
        meta[o + C + S:o + C + 2 * S, 0] = dsend_all[c * S:(c + 1) * S]
        meta[o + C + 2 * S:o + 2 * C + 2 * S] = grecv[c * C:(c + 1) * C]
        meta[o + 2 * C + 2 * S:o + NMETA, 0] = wbits[c * S:(c + 1) * S]

    args = {"xsr": xsr_dev, "meta": meta, **wdev}
    ordered = [args[n] if n == "xsr" or not isinstance(args[n], np.ndarray)
               else jax.device_put(args[n], shard) for n in ctx["in_names"]]
    ordered.extend(ctx["out_operands"])
    for _ in range(ITERS - 1):   # extra device executions for timing
        ctx["fn"](*ordered)
    res = ctx["fn"](*ordered)
    return np.asarray(res[0]).astype(np.float32)


# revision 4
# speedup vs baseline: 13.3167x; 1.4577x over previous
"""Llama4 MoE (T=4096 H=2048 I=1024 E=16 top-1) on 8 trn2 cores, expert-parallel.

v4: minimal client<->device traffic + minimal operand count.
  - Upload per call: hidden_states bf16 [T,H] (sharded by token slice) and a
    packed i32 meta tensor (indices + router-weight bits). ~16.1MB total,
    overlapped with host-side index building (device_put is async).
  - Device: scale tokens, AllToAll token dispatch to expert-owning cores,
    PE-transpose, expert+shared GatedMLPs, bf16 AllToAll combine, final add.
    Output is bf16 (halves result-staging + download).
  - All weights live in ONE device-resident bf16 blob, built once and
    fingerprint-cached. The jitted shard_map executable is cached too.
  - If routing exceeds the baked capacities, falls back to exact numpy.

Device program (per core c, owning experts 2c/2c+1):
  xsr rows -> scale by sigmoid(router) -> indirect-scatter to send2 blocked
  by expert core -> AllToAll -> (shared-expert GEMMs overlap) ->
  indirect-gather packed expert tokens -> PE transpose -> expert GEMMs ->
  indirect-scatter bf16 y rows to send blocked by home core -> AllToAll ->
  indirect-gather + add -> bf16 out.
"""
import numpy as np
import ml_dtypes

import jax
from jax.sharding import Mesh, PartitionSpec, NamedSharding
from jax.experimental.shard_map import shard_map

import concourse.bass as bass
import concourse.mybir as mybir
import concourse.tile as tile
from concourse import bacc, bass2jax
from concourse.bass2jax import _bass_exec_p, partition_id_tensor
from concourse.masks import make_identity

T, H, I, E = 4096, 2048, 1024, 16
NCORES = 8
S = T // NCORES          # 512 tokens per slice
EPC = E // NCORES        # 2 experts per core
CE = 384                 # per-expert token capacity (3 tiles of 128)
C = EPC * CE             # 768 gathered tokens per core
B = 96                   # AllToAll rows per (src,dst) block
NB = NCORES * B          # 768 rows in send/recv buffers
KT = H // 128            # 16 contraction tiles over H
MT_S = S // 128          # 4 token tiles per slice
MT_E = CE // 128         # 3 token tiles per expert
GT = C // 128            # 6 gathered-token tiles per core
NMETA = C + S + S + C + S   # sidx | gidx | dsend | grecv | wg-bits
WROWS = 9 * H            # weight blob rows (width I)
F32 = mybir.dt.float32
BF16 = mybir.dt.bfloat16
I32 = mybir.dt.int32

_CACHE = {}
ITERS = 1
_BF = ml_dtypes.bfloat16


def _build():
    nc = bacc.Bacc("TRN2", target_bir_lowering=False, debug=False,
                   enable_asserts=False, num_devices=NCORES)

    xsr = nc.dram_tensor("xsr", [S, H], BF16, kind="ExternalInput").ap()
    meta = nc.dram_tensor("meta", [NMETA, 1], I32, kind="ExternalInput").ap()
    wb = nc.dram_tensor("wb", [WROWS, I], BF16, kind="ExternalInput").ap()
    out = nc.dram_tensor("out", [S, H], BF16, kind="ExternalOutput").ap()

    def up_view(r0):       # [H, I] matrix at blob row r0
        return wb[r0:r0 + H, :]

    def down_view(r0):     # [I, 2048] matrix stored as [2048, 1024] rows
        return wb[r0:r0 + H, :].rearrange("(p q) w -> p (q w)", q=2)

    ew1 = [up_view(el * H) for el in range(EPC)]
    ew3 = [up_view((2 + el) * H) for el in range(EPC)]
    ew2 = [down_view((4 + el) * H) for el in range(EPC)]
    sw1 = up_view(6 * H)
    sw3 = up_view(7 * H)
    sw2 = down_view(8 * H)

    with tile.TileContext(nc) as tc:
        with (
            tc.tile_pool(name="persist", bufs=1) as pp,
            tc.tile_pool(name="hpool", bufs=1) as hp,
            tc.tile_pool(name="ypool", bufs=3) as yp,
            tc.tile_pool(name="rpool", bufs=2) as rp,
            tc.tile_pool(name="stream", bufs=3) as sp,
            tc.tile_pool(name="wdpool", bufs=10) as wdp,
            tc.tile_pool(name="xrow", bufs=1) as xrp,
            tc.tile_pool(name="psum", bufs=1, space="PSUM") as psp,
            tc.tile_pool(name="dram", bufs=1, space="DRAM") as dp,
        ):
            send2 = dp.tile([NB, H], BF16, tag="send2")   # dispatch
            recv2 = dp.tile([NB, H], BF16, tag="recv2")
            send = dp.tile([NB, H], BF16, tag="send")     # combine
            recv = dp.tile([NB, H], BF16, tag="recv")

            for _it in range(ITERS):
                # ---- unpack meta: sidx | gidx | dsend | grecv | wg ----
                mview = meta.rearrange("(m p) one -> m p one", p=128)
                SIDX = pp.tile([128, GT], I32, tag="sidx")
                for m in range(GT):
                    nc.sync.dma_start(SIDX[:, m:m + 1], mview[m])
                GIDX = pp.tile([128, MT_S], I32, tag="gidx")
                for m in range(MT_S):
                    nc.sync.dma_start(GIDX[:, m:m + 1], mview[GT + m])
                DSX = pp.tile([128, MT_S], I32, tag="dsx")
                for m in range(MT_S):
                    nc.sync.dma_start(DSX[:, m:m + 1], mview[GT + MT_S + m])
                GRX = pp.tile([128, GT], I32, tag="grx")
                for m in range(GT):
                    nc.sync.dma_start(GRX[:, m:m + 1],
                                      mview[GT + 2 * MT_S + m])
                WGS = pp.tile([128, MT_S], F32, tag="wgs")
                for m in range(MT_S):
                    nc.sync.dma_start(WGS[:, m:m + 1],
                                      mview[2 * GT + 2 * MT_S + m].bitcast(F32))

                IDN = pp.tile([128, 128], BF16, tag="idn")
                make_identity(nc, IDN[:])

                # ---- dispatch: load slice rows, scale, scatter to send2 ----
                xss = [xrp.tile([128, H], BF16, tag=f"xr{g}", name=f"xr{g}")
                       for g in range(GT)]      # xr0-3: slice rows; reused
                for g in range(MT_S):
                    nc.sync.dma_start(xss[g][:],
                                      xsr[g * 128:(g + 1) * 128, :])
                for g in range(MT_S):
                    xsc = rp.tile([128, H], BF16, tag="xsc", name="xsc")
                    nc.vector.tensor_scalar_mul(xsc[:], xss[g][:],
                                                WGS[:, g:g + 1])
                    nc.gpsimd.indirect_dma_start(
                        out=send2[:],
                        out_offset=bass.IndirectOffsetOnAxis(
                            ap=DSX[:, g:g + 1], axis=0),
                        in_=xsc[:], in_offset=None)
                nc.gpsimd.collective_compute(
                    "AllToAll", mybir.AluOpType.bypass,
                    replica_groups=[list(range(NCORES))],
                    ins=[send2[:].opt()], outs=[recv2[:].opt()])

                # ---- PE transpose slice rows for the shared expert ----
                # XS[p, k*S + t] = x[token t, H k*128+p]
                XS = pp.tile([128, KT * S], BF16, tag="xs")
                for k in range(KT):
                    pt = psp.tile([128, S], BF16, tag="ptr", space="PSUM")
                    for g in range(MT_S):
                        nc.tensor.transpose(
                            pt[:, g * 128:(g + 1) * 128],
                            xss[g][:, k * 128:(k + 1) * 128], IDN[:])
                    nc.vector.tensor_copy(XS[:, k * S:(k + 1) * S], pt[:])

                # ---- gated MLP ----
                def gated_mlp(xtile, xoff, nmt, xstride, w1d, w3d, w2d, ysink):
                    ntok = nmt * 128
                    HH = []
                    for mat, wd in ((0, w1d), (1, w3d)):
                        HT = hp.tile([128, 8 * ntok], BF16, tag=f"h{mat}_{ntok}")
                        for half in range(2):
                            pus = [psp.tile([128, ntok], F32, tag=f"pu{i}",
                                            name=f"pu{i}", space="PSUM")
                                   for i in range(4)]
                            for k in range(KT):
                                wp = sp.tile([128, 512], BF16, tag="wup")
                                nc.sync.dma_start(
                                    wp[:], wd[k * 128:(k + 1) * 128,
                                              half * 512:(half + 1) * 512])
                                for i in range(4):
                                    nc.tensor.matmul(
                                        pus[i][:],
                                        wp[:, i * 128:(i + 1) * 128],
                                        xtile[:, k * xstride + xoff:
                                              k * xstride + xoff + ntok],
                                        start=(k == 0), stop=(k == KT - 1))
                            for i in range(4):
                                it = half * 4 + i
                                nc.vector.tensor_copy(
                                    HT[:, it * ntok:(it + 1) * ntok], pus[i][:])
                        HH.append(HT)
                    H1, H3 = HH
                    nc.scalar.activation(H1[:], H1[:],
                                         mybir.ActivationFunctionType.Silu)
                    nc.vector.tensor_mul(H1[:], H1[:], H3[:])
                    for half in range(2):
                        wps = [wdp.tile([128, 1024], BF16, tag="wdn", name="wdn")
                               for _ in range(8)]
                        for k in range(8):
                            nc.sync.dma_start(
                                wps[k][:], w2d[k * 128:(k + 1) * 128,
                                               half * 1024:(half + 1) * 1024])
                        for m in range(nmt):
                            for n2 in range(2):
                                pd = psp.tile([128, 512], F32, tag=f"pd{m % 3}",
                                              space="PSUM")
                                for k in range(8):
                                    nc.tensor.matmul(
                                        pd[:],
                                        H1[:, k * ntok + m * 128:
                                           k * ntok + (m + 1) * 128],
                                        wps[k][:, n2 * 512:(n2 + 1) * 512],
                                        start=(k == 0), stop=(k == 7))
                                ysink(m, half * 1024 + n2 * 512, pd)

                # ---- shared expert first: overlaps the dispatch AllToAll ----
                YS = [pp.tile([128, H], F32, tag=f"ys{m}", name=f"ys{m}")
                      for m in range(MT_S)]

                def shared_sink(off):
                    def sink(m, col, pd):
                        nc.vector.tensor_copy(YS[off + m][:, col:col + 512],
                                              pd[:])
                    return sink

                # first half of the shared expert overlaps the dispatch A2A
                gated_mlp(XS, 0, MT_S // 2, S, sw1, sw3, sw2, shared_sink(0))

                # ---- gather dispatched tokens, transpose to XG ----
                for g in range(GT):
                    nc.gpsimd.indirect_dma_start(
                        out=xss[g][:], out_offset=None, in_=recv2[:],
                        in_offset=bass.IndirectOffsetOnAxis(
                            ap=GRX[:, g:g + 1], axis=0))
                XG = pp.tile([128, KT * C], BF16, tag="xg")
                for k in range(KT):
                    pt = psp.tile([128, C], BF16, tag="ptr", space="PSUM")
                    for g in range(GT):
                        nc.tensor.transpose(
                            pt[:, g * 128:(g + 1) * 128],
                            xss[g][:, k * 128:(k + 1) * 128], IDN[:])
                    nc.vector.tensor_copy(XG[:, k * C:(k + 1) * C], pt[:])

                # ---- routed experts: bf16 y rows -> scatter to send ----
                YT = {}

                def routed_sink(el):
                    def sink(m, col, pd):
                        key = (el, m)
                        if key not in YT:
                            YT[key] = yp.tile([128, H], BF16, tag="yrow",
                                              name="yrow")
                        nc.vector.tensor_copy(YT[key][:, col:col + 512], pd[:])
                        if col == H - 512:
                            gm = el * MT_E + m
                            nc.gpsimd.indirect_dma_start(
                                out=send[:],
                                out_offset=bass.IndirectOffsetOnAxis(
                                    ap=SIDX[:, gm:gm + 1], axis=0),
                                in_=YT.pop(key)[:], in_offset=None)
                    return sink

                for el in range(EPC):
                    gated_mlp(XG, el * CE, MT_E, C, ew1[el], ew3[el], ew2[el],
                              routed_sink(el))

                # ---- combine: AllToAll + gather + add + store (bf16) ----
                nc.gpsimd.collective_compute(
                    "AllToAll", mybir.AluOpType.bypass,
                    replica_groups=[list(range(NCORES))],
                    ins=[send[:].opt()], outs=[recv[:].opt()])
                # second half of the shared expert overlaps the combine A2A
                gated_mlp(XS, (MT_S // 2) * 128, MT_S // 2, S,
                          sw1, sw3, sw2, shared_sink(MT_S // 2))
                for m in range(MT_S):
                    rg = rp.tile([128, H], BF16, tag="rg")
                    nc.gpsimd.indirect_dma_start(
                        out=rg[:], out_offset=None, in_=recv[:],
                        in_offset=bass.IndirectOffsetOnAxis(
                            ap=GIDX[:, m:m + 1], axis=0))
                    ob = rp.tile([128, H], BF16, tag="ob")
                    nc.vector.tensor_add(ob[:], YS[m][:], rg[:])
                    nc.sync.dma_start(out[m * 128:(m + 1) * 128, :], ob[:])

    nc.compile()
    return nc


def _make_exec(nc):
    """Build the cached jitted shard_map executable."""
    bass2jax.install_neuronx_cc_hook()
    assert nc.dbg_addr is None

    partition_name = (nc.partition_id_tensor.name
                      if nc.partition_id_tensor else None)
    in_names, out_names, out_avals = [], [], []
    for alloc in nc.m.functions[0].allocations:
        if not isinstance(alloc, mybir.MemoryLocationSet):
            continue
        name = alloc.memorylocations[0].name
        if alloc.kind == "ExternalInput":
            if name != partition_name:
                in_names.append(name)
        elif alloc.kind == "ExternalOutput":
            out_names.append(name)
            out_avals.append(jax.core.ShapedArray(tuple(alloc.tensor_shape),
                                                  mybir.dt.np(alloc.dtype)))
    all_names = in_names + out_names

    devices = jax.devices()[:NCORES]
    mesh = Mesh(np.asarray(devices), ("core",))
    shard = NamedSharding(mesh, PartitionSpec("core"))

    def _body(*args):
        operands = list(args)
        if partition_name is not None:
            operands.append(partition_id_tensor())
        outs = _bass_exec_p.bind(
            *operands,
            out_avals=tuple(out_avals),
            in_names=tuple(all_names + ([partition_name]
                                        if partition_name else [])),
            out_names=tuple(out_names),
            lowering_input_output_aliases=(),
            sim_require_finite=True,
            sim_require_nnan=True,
            nc=nc,
        )
        return tuple(outs)

    fn = jax.jit(
        shard_map(_body, mesh=mesh,
                  in_specs=(PartitionSpec("core"),) * len(all_names),
                  out_specs=(PartitionSpec("core"),) * len(out_names),
                  check_rep=False),
        keep_unused=True,
    )
    # persistent non-donated operands for the ExternalOutput slots (the NEFF
    # writes every element of `out`, so their initial value is never observed)
    out_operands = [
        jax.device_put(np.zeros((NCORES * av.shape[0],) + av.shape[1:],
                                av.dtype), shard)
        for av in out_avals]
    return {"fn": fn, "in_names": in_names, "shard": shard,
            "out_operands": out_operands}


def _prep_weights(ctx, ws):
    """Pack all weights into one bf16 blob per core, device-resident once."""
    key = tuple(
        (w.shape, w.dtype.str, bytes(np.ascontiguousarray(
            np.asarray(w).ravel()[::4099][:64]).data))
        for w in ws)
    if _CACHE.get("wkey") == key:
        return _CACHE["wdev"]
    shared_w1, shared_w3, shared_w2, expert_w1, expert_w3, expert_w2 = (
        np.ascontiguousarray(np.asarray(w, dtype=np.float32)) for w in ws)

    blob = np.empty((NCORES * WROWS, I), _BF)
    s1 = shared_w1.astype(_BF)
    s3 = shared_w3.astype(_BF)
    s2 = shared_w2.reshape(H, I).astype(_BF)
    for c in range(NCORES):
        o = c * WROWS
        for el in range(EPC):
            e = c * EPC + el
            blob[o + el * H:o + (el + 1) * H] = expert_w1[e].astype(_BF)
            blob[o + (2 + el) * H:o + (3 + el) * H] = expert_w3[e].astype(_BF)
            blob[o + (4 + el) * H:o + (5 + el) * H] = (
                expert_w2[e].reshape(H, I).astype(_BF))
        blob[o + 6 * H:o + 7 * H] = s1
        blob[o + 7 * H:o + 8 * H] = s3
        blob[o + 8 * H:o + 9 * H] = s2
    wdev = {"wb": jax.device_put(blob, ctx["shard"])}
    _CACHE["wkey"] = key
    _CACHE["wdev"] = wdev
    return wdev


def _numpy_fallback(hs, rw, sw1, sw3, sw2, ew1, ew3, ew2):
    """Exact fp32 reference math (used only if routing capacity is exceeded)."""
    def silu(x):
        return x / (1.0 + np.exp(-x))

    def gmlp(x, w1, w3, w2):
        return (silu(x @ w1) * (x @ w3)) @ w2

    shared = gmlp(hs, sw1, sw3, sw2)
    logits = hs @ rw
    top = logits.argmax(1)
    w = 1.0 / (1.0 + np.exp(-logits[np.arange(T), top]))
    routed = np.zeros_like(shared)
    for e in range(E):
        tk = np.flatnonzero(top == e)
        if len(tk):
            xe = hs[tk] * w[tk, None]
            routed[tk] = gmlp(xe, ew1[e], ew3[e], ew2[e])
    return shared + routed


def kernel(hidden_states, router_w, shared_w1, shared_w3, shared_w2,
           expert_w1, expert_w3, expert_w2):
    if "nc" not in _CACHE:
        _CACHE["nc"] = _build()
        _CACHE["ctx"] = _make_exec(_CACHE["nc"])
    ctx = _CACHE["ctx"]
    wdev = _prep_weights(ctx, (shared_w1, shared_w3, shared_w2,
                               expert_w1, expert_w3, expert_w2))

    hs = np.ascontiguousarray(np.asarray(hidden_states, dtype=np.float32))
    rw = np.ascontiguousarray(np.asarray(router_w, dtype=np.float32))
    shard = ctx["shard"]

    # start the big upload first; index math below overlaps with it
    xsr_dev = jax.device_put(hs.astype(_BF), shard)

    logits = hs @ rw
    top = logits.argmax(1)
    wtok = (1.0 / (1.0 + np.exp(-logits[np.arange(T), top]))).astype(np.float32)
    toks = [np.flatnonzero(top == e) for e in range(E)]
    if max(len(t) for t in toks) > CE:
        return _numpy_fallback(
            hs, rw,
            *(np.ascontiguousarray(np.asarray(w, dtype=np.float32)) for w in
              (shared_w1, shared_w3, shared_w2, expert_w1, expert_w3,
               expert_w2)))

    # dispatch indices: send row (home side) and recv position per token
    ecore = top // EPC
    dsend_all = np.empty(T, np.int64)
    pos2_tok = np.empty(T, np.int64)
    ok = True
    for c in range(NCORES):
        tkc = np.flatnonzero(ecore == c)          # sorted by token id
        d = tkc // S                              # nondecreasing
        starts = np.searchsorted(tkc, np.arange(NCORES) * S)
        pos2 = np.arange(len(tkc)) - starts[d]
        if pos2.max(initial=0) > B - 2:
            ok = False
            break
        dsend_all[tkc] = c * B + pos2
        pos2_tok[tkc] = pos2
    if not ok:
        return _numpy_fallback(
            hs, rw,
            *(np.ascontiguousarray(np.asarray(w, dtype=np.float32)) for w in
              (shared_w1, shared_w3, shared_w2, expert_w1, expert_w3,
               expert_w2)))

    # combine indices (identical counts to dispatch, reversed direction)
    sidx = np.empty((NCORES * C, 1), np.int32)
    grecv = np.zeros((NCORES * C, 1), np.int32)
    gidx_all = np.zeros(T, np.int32)
    for c in range(NCORES):
        send_idx = np.full(C, c * B + B - 1, np.int64)  # pads -> dump row
        pos_d = [0] * NCORES
        for el in range(EPC):
            tk = toks[c * EPC + el]
            r0 = c * C + el * CE
            if len(tk):
                grecv[r0:r0 + len(tk), 0] = (tk // S) * B + pos2_tok[tk]
            d = tk // S
            for dd in range(NCORES):
                sel = np.flatnonzero(d == dd)
                if not len(sel):
                    continue
                p0 = pos_d[dd]
                p = p0 + np.arange(len(sel))
                send_idx[el * CE + sel] = dd * B + p
                gidx_all[tk[sel]] = c * B + p
                pos_d[dd] = p0 + len(sel)
        sidx[c * C:(c + 1) * C, 0] = send_idx

    meta = np.empty((NCORES * NMETA, 1), np.int32)
    wbits = wtok.view(np.int32)
    for c in range(NCORES):
        o = c * NMETA
        meta[o:o + C] = sidx[c * C:(c + 1) * C]
        meta[o + C:o + C + S, 0] = gidx_all[c * S:(c + 1) * S]
        meta[o + C + S:o + C + 2 * S, 0] = dsend_all[c * S:(c + 1) * S]
        meta[o + C + 2 * S:o + 2 * C + 2 * S] = grecv[c * C:(c + 1) * C]
        meta[o + 2 * C + 2 * S:o + NMETA, 0] = wbits[c * S:(c + 1) * S]

    args = {"xsr": xsr_dev, "meta": meta, **wdev}
    ordered = [args[n] if n == "xsr" or not isinstance(args[n], np.ndarray)
               else jax.device_put(args[n], shard) for n in ctx["in_names"]]
    ordered.extend(ctx["out_operands"])
    for _ in range(ITERS - 1):   # extra device executions for timing
        ctx["fn"](*ordered)
    res = ctx["fn"](*ordered)
    return np.asarray(res[0]).astype(np.float32)
